# revision 18
# baseline (speedup 1.0000x reference)
"""Multi-head causal attention (B=2, L=2048, D=1024, H=16) on 8 TRN2 cores.

Sharding: data-parallel over batch (cores 0-3 -> b=0, cores 4-7 -> b=1),
tensor-parallel over heads (each core computes 4 of the 16 heads).  Each core
returns a partial [L, D] output-projection contribution; the host sums the
4 partials per batch and adds bo.

Per-core kernel, engineered around the PE clock ramp (the tensor engine
needs ~3us of gapless execution to reach 2.4GHz; any stall drops it back
to 1.2GHz), so the whole kernel aims to be one continuous PE stream:

  - bf16 matmul inputs everywhere (weights cast on SWDGE load; x windows
    loaded f32 on the ACT HWDGE queue, PE-transposed, cast to bf16 during
    the PSUM evacuation).  PSUM accumulation stays f32.
  - causal trimming: for diagonal k-blocks only the valid query range
    [128*s, 512) is computed by S / exp / PV; the 128-wide triangle is
    masked in PSUM (fill=-1e30, exp -> 0) by gpsimd affine_select BEFORE
    the exp, so P needs no post-masking.
  - kt order per head-pair is diagonal-blocks-first so the start=True
    PV accumulation always covers the full query range.
  - PV places the odd head at PSUM partitions 63..127 (ones column first,
    denominator at row 63) so normalized O^T rows land on partitions
    64..127 without any partition-hop DMA.  The whole pair is evacuated
    bf16 to SBUF immediately (frees the banks), then reciprocal /
    partition_broadcast / tensor_mul run as deferred queue work.
  - a deficit ledger paces the attention inner loop: each kt unit's exp
    time is compared against its S+PV PE time and the gap is filled by
    pulling next-window prep work (transposes, QKV projections) and
    previous-window output-projection units into the PE stream.
  - PSUM budget (8 banks): 2x[128,2,512] S tiles (4) + 3x[128,512] PV
    accum (3) + 1x[128,512] prep (1).  Pure-prep phases (head, window
    boundaries) double-buffer prep groups by borrowing idle S-pool tiles.
  - DMA: x loads on qAct (dep-free triggers, never stall the ACT stream),
    output stores on qSP, weight casts on SWDGE.
"""

import math
from collections import deque

import numpy as np

import concourse.bass as bass
import concourse.tile as tile
from concourse import bacc, mybir
from concourse.bass_utils import run_bass_kernel_spmd
from concourse.vector_clock import VectorClock, ScopedClock

F32 = mybir.dt.float32
F32R = mybir.dt.float32r
BF16 = mybir.dt.bfloat16

B, L, D, H = 2, 2048, 1024, 16
DKH = 64          # head dim
HC = 4            # heads per core
DKC = HC * DKH    # 256 projected cols per core
LW = 512          # query window
NW = L // LW      # 4 windows
NKT = L // 128    # 16 k tiles

MMDT = BF16       # matmul input dtype
TPDT = F32R       # transpose path dtype (x loaded f32, cast at evac)

NEG = -1.0e30     # mask fill; exp -> 0


class _SplitDrainTileContext(tile.TileContext):
    """The walrus build in this container only supports a single sync-wait
    per Drain instruction; split the kernel-tail drain into one drain per
    outstanding semaphore."""

    def _drain_and_barrier(self, tick_clock, wait_clock):
        gc = tick_clock.global_clock
        n = len(gc)
        active = [i for i in range(n) if gc[i] > 0]
        for i in active:
            vc = VectorClock([gc[j] if j == i else 0 for j in range(n)])
            di = self.nc.sync.drain()
            wait_clock.add_sem_waits(di.ins, ScopedClock({None: vc}))
        self.nc.all_engine_barrier()
        popped = self.nc._tile_sem_poison_stack.pop()
        assert popped is self._sem_poison
        self.nc.clear_and_free_semaphores(list(self.sems.allocated().values()))
        self.nc.all_engine_barrier()


def build_program() -> bass.Bass:
    nc = bacc.Bacc("TRN2", target_bir_lowering=False, debug=False)

    x_ctx = nc.declare_dram_parameter("x_ctx", [L, D], F32, isOutput=False)
    x_val = nc.declare_dram_parameter("x_val", [L, D], F32, isOutput=False)
    wq = nc.declare_dram_parameter("wq", [D, DKC], F32, isOutput=False)
    wk = nc.declare_dram_parameter("wk", [D, DKC], F32, isOutput=False)
    wv = nc.declare_dram_parameter("wv", [D, DKC], F32, isOutput=False)
    bq = nc.declare_dram_parameter("bq", [DKC], F32, isOutput=False)
    bk = nc.declare_dram_parameter("bk", [DKC], F32, isOutput=False)
    bv = nc.declare_dram_parameter("bv", [DKC], F32, isOutput=False)
    wo = nc.declare_dram_parameter("wo", [DKC, D], F32, isOutput=False)
    ident_in = nc.declare_dram_parameter("ident", [128, 128], F32, isOutput=False)
    out = nc.declare_dram_parameter("out", [L, D], F32, isOutput=True)

    with _SplitDrainTileContext(nc) as tc:
        with (
            tc.tile_pool(name="consts", bufs=1) as consts,
            tc.tile_pool(name="resident", bufs=1) as resident,
            tc.tile_pool(name="xraw", bufs=4) as xraw_pool,
            tc.tile_pool(name="xT", bufs=2) as xT_pool,
            tc.tile_pool(name="qT", bufs=2) as qT_pool,
            tc.tile_pool(name="pp", bufs=3) as p_pool,
            tc.tile_pool(name="norm", bufs=8) as norm_pool,
            tc.tile_pool(name="ost", bufs=4) as ost_pool,
            tc.tile_pool(name="ps_s", bufs=2, space="PSUM") as ps_s,
            tc.tile_pool(name="ps_o", bufs=3, space="PSUM") as ps_o,
            tc.tile_pool(name="ps_p", bufs=1, space="PSUM") as ps_p,
        ):
            # ---- t=0: warm the ACT exp table (one-time ~2.7us load) ----
            warm = consts.tile([1, 8], F32, tag="warm")
            nc.vector.memset(warm[:, :], 0.0)
            nc.scalar.activation(warm[:, :], warm[:, :],
                                 func=mybir.ActivationFunctionType.Exp)

            # ---- loads: x windows on qAct (dep-free triggers), weight
            # casts on SWDGE, small stuff on qSP ----
            ident = consts.tile([128, 128], TPDT, tag="ident")
            nc.sync.dma_start(out=ident[:, :], in_=ident_in[:, :].bitcast(TPDT))

            xq = {}

            def load_window(src_dram, key, lw_):
                xw = xraw_pool.tile([128, 4, D], TPDT, tag="xraw")
                lsl_ = slice(lw_ * LW, (lw_ + 1) * LW)
                nc.scalar.dma_start(
                    out=xw[:, :, :],
                    in_=src_dram[lsl_, :].rearrange("(a p) d -> p a d", p=128).bitcast(TPDT))
                xq[(key, lw_)] = xw

            load_window(x_ctx, 'c', 0)
            load_window(x_val, 'v', 0)

            wq_sb = consts.tile([128, 8, DKC], MMDT, tag="wq")
            nc.gpsimd.dma_start(out=wq_sb[:, :, :],
                                in_=wq[:, :].rearrange("(k p) n -> p k n", p=128))
            wk_sb = consts.tile([128, 8, DKC], MMDT, tag="wk")
            nc.gpsimd.dma_start(out=wk_sb[:, :, :],
                                in_=wk[:, :].rearrange("(k p) n -> p k n", p=128))
            wv_sb = consts.tile([128, 8, DKC], MMDT, tag="wv")
            nc.gpsimd.dma_start(out=wv_sb[:, :, :],
                                in_=wv[:, :].rearrange("(k p) n -> p k n", p=128))
            wo_sb = consts.tile([128, 2, D], MMDT, tag="wo")
            nc.gpsimd.dma_start(out=wo_sb[:, :, :],
                                in_=wo[:, :].rearrange("(m p) n -> p m n", p=128))

            bq_sb = consts.tile([128, 2], F32, tag="bq")
            nc.sync.dma_start(out=bq_sb[:, :], in_=bq[:].rearrange("(m p) -> p m", p=128))
            bk_sb = consts.tile([128, 2], F32, tag="bk")
            nc.sync.dma_start(out=bk_sb[:, :], in_=bk[:].rearrange("(m p) -> p m", p=128))
            bv_bc = consts.tile([128, DKC], F32, tag="bv")
            nc.sync.dma_start(
                out=bv_bc[:, :], in_=bv[:].unsqueeze(0).broadcast_to([128, DKC]))

            # ---- resident tensors ----
            # kT: head h of pair hp=h//2 lives at partitions (h%2)*64..+64,
            # free dims [m=hp, l].
            kT_sb = resident.tile([128, 2, L], MMDT, tag="kT")
            # v: per (ktile, head) a 65-wide group [V | ones]; the ones
            # column accumulates the softmax denominator at PV row 64.
            v_sb = resident.tile([128, NKT, HC, 1 + DKH], MMDT, tag="v")
            oT_sb = resident.tile([128, 2, L], MMDT, tag="oT")

            nc.vector.memset(v_sb[:, :, :, DKH:DKH + 1], 1.0)

            # ================= scheduler machinery =================
            # Closures in prep_q/fill_q each emit ONE small group of
            # instructions (a few PE ops + their evacuation) and return
            # their nominal PE-time in ns (or None to defer: "I can't run
            # under no_pool").  The attention loop pulls from them to keep
            # the PE busy while ACT chews on exp.  no_pool=True keeps the
            # pulled work off the GpSimd stream so a pending diagonal-mask
            # affine_select is never delayed behind it.
            prep_q = deque()
            fill_q = deque()
            state = {"rr": 0, "attn": False, "act_free": True, "pp_rr": 0, "act_debt": 0.0}

            def prep_psum():
                # During attention the single ps_p bank is enough (groups
                # are spaced out by exp units).  In pure-prep phases,
                # double-buffer by borrowing the idle S pool's (2-bank)
                # tiles for every other group.
                if state["attn"]:
                    return ps_p.tile([128, LW], F32, tag="prep", name="prep")
                state["pp_rr"] += 1
                if state["pp_rr"] % 2:
                    return ps_p.tile([128, LW], F32, tag="prep", name="prep")
                return ps_s.tile([128, 2, LW], F32, tag="s", name="sborrow")[:, 0, :]

            def evac_copy(dst, src, no_pool=False):
                # PSUM evacuation: GPSIMD cannot access PSUM, so only DVE
                # and ACT qualify.  ACT takes every 3rd copy; during
                # attention that delays exp, which the ledger accounts for
                # via act_debt.
                state["rr"] += 1
                opts = [nc.vector, nc.vector, None] if state["act_free"] \
                    else [nc.vector, nc.vector, None]
                eng = opts[state["rr"] % len(opts)]
                if eng is None:
                    if not state["act_free"]:
                        state["act_debt"] += 850
                    nc.scalar.activation(dst, src,
                                         func=mybir.ActivationFunctionType.Copy)
                else:
                    eng.tensor_copy(dst, src)

            def pull_one(ns_budget, no_pool=False):
                """Emit at most one queued group if the ledger says the PE
                has slack.  Returns the remaining budget."""
                if ns_budget <= 0:
                    return ns_budget
                q = prep_q if prep_q else fill_q
                if not q:
                    return ns_budget
                r = q[0](no_pool)
                if r is None:
                    return ns_budget  # deferred (needs the Pool engine)
                q.popleft()
                return ns_budget - r

            def drain_prep():
                while prep_q:
                    r = prep_q[0](False)
                    assert r is not None
                    prep_q.popleft()

            # ================= prep groups =================
            def t_group(key, lw_, k, xT):
                """Transpose x[:, k*128:+128] of one window into xT[:, k, :]."""
                def run(no_pool=False):
                    xw = xq[(key, lw_)]
                    pt = prep_psum()
                    for a in range(4):
                        nc.tensor.transpose(
                            pt[:, a * 128:(a + 1) * 128].bitcast(TPDT),
                            xw[:, a, k * 128:(k + 1) * 128], ident[:, :])
                    evac_copy(xT[:, k, :], pt[:, :], no_pool)
                    if k == 7:
                        xq.pop((key, lw_))
                    return 4 * 80
                return run

            def qk_group(which, m, lw_, xT, qT):
                """One 128-row chunk of Q^T or K^T for window lw_."""
                def run(no_pool=False):
                    w_sb = wq_sb if which == 'q' else wk_sb
                    b_sb = bq_sb if which == 'q' else bk_sb
                    pq = prep_psum()
                    for k in range(8):
                        nc.tensor.matmul(
                            pq[:, :],
                            w_sb[:, k, m * 128:(m + 1) * 128],
                            xT[:, k, :],
                            start=(k == 0), stop=(k == 7),
                        )
                    lsl_ = slice(lw_ * LW, (lw_ + 1) * LW)
                    dst = qT[:, m, :] if which == 'q' else kT_sb[:, m, lsl_]
                    nc.vector.tensor_scalar_add(dst, pq[:, :], b_sb[:, m:m + 1])
                    return 8 * 213
                return run

            def v_group(lw_, a, xvT):
                """One 128-token chunk of V (all 4 heads) for window lw_."""
                def run(no_pool=False):
                    pv = prep_psum()
                    for k in range(8):
                        nc.tensor.matmul(
                            pv[:, 0:DKC],
                            xvT[:, k, a * 128:(a + 1) * 128],
                            wv_sb[:, k, :],
                            start=(k == 0), stop=(k == 7),
                        )
                    kt_abs = lw_ * 4 + a
                    eng = nc.vector  # PSUM input: GPSIMD can't access PSUM
                    pvh = pv[:, 0:DKC].rearrange("p (h d) -> p h d", h=HC)
                    bvh = bv_bc[:, :].rearrange("p (h d) -> p h d", h=HC)
                    eng.tensor_add(
                        v_sb[:, kt_abs, :, 0:DKH], pvh[:, :, :], bvh[:, :, :])
                    return 8 * 107
                return run

            def outproj_unit(lt, n):
                """One [128, 512] tile of partial = O^T.T @ Wo, stored on qSP."""
                def run(no_pool=False):
                    pop = prep_psum()
                    for m in range(2):
                        nc.tensor.matmul(
                            pop[:, :],
                            oT_sb[:, m, lt * 128:(lt + 1) * 128],
                            wo_sb[:, m, n * 512:(n + 1) * 512],
                            start=(m == 0), stop=(m == 1),
                        )
                    ost = ost_pool.tile([128, LW], F32, tag="ost")
                    evac_copy(ost[:, :], pop[:, :], no_pool)
                    nc.sync.dma_start(
                        out=out[lt * 128:(lt + 1) * 128, n * 512:(n + 1) * 512],
                        in_=ost[:, :],
                    )
                    return 2 * 213
                return run

            def enqueue_prep(lw_, xT, xvT, qT):
                for k in range(8):
                    prep_q.append(t_group('c', lw_, k, xT))
                for m, which in ((0, 'q'), (0, 'k'), (1, 'q'), (1, 'k')):
                    prep_q.append(qk_group(which, m, lw_, xT, qT))
                for k in range(8):
                    prep_q.append(t_group('v', lw_, k, xvT))
                for a in range(4):
                    prep_q.append(v_group(lw_, a, xvT))

            # ================= attention =================
            def norm_chain(hp, lsl_, po_e, po_o):
                """Evacuate one head pair's PV PSUM (frees the banks now),
                queue the normalize as prep pieces.  partition_broadcast
                reads physical partition 0, so the denominator rows are
                first hopped down to partitions 0/1 via SWDGE."""
                stg_e = norm_pool.tile([128, LW], MMDT, tag="stg")
                stg_o = norm_pool.tile([128, LW], MMDT, tag="stg")
                nc.vector.tensor_copy(stg_e[0:65, :], po_e[0:65, :])
                nc.vector.tensor_copy(stg_o[0:65, :], po_o[0:65, :])
                rden = norm_pool.tile([2, LW], MMDT, tag="rden")
                rrec = norm_pool.tile([2, LW], MMDT, tag="rrec")
                rr1 = norm_pool.tile([1, LW], MMDT, tag="rr1")
                rb_e = norm_pool.tile([64, LW], MMDT, tag="rb")
                rb_o = norm_pool.tile([64, LW], MMDT, tag="rb")
                onorm = norm_pool.tile([64, LW], MMDT, tag="onorm")

                def hops_den(no_pool=False):
                    if no_pool:
                        return None  # SWDGE trigger would block the Pool stream
                    nc.gpsimd.dma_start(out=rden[0:1, :], in_=stg_e[64:65, :])
                    nc.gpsimd.dma_start(out=rden[1:2, :], in_=stg_o[64:65, :])
                    return 0

                def recips(no_pool=False):
                    with nc.allow_low_precision(
                            reason="bf16 softmax denom reciprocal; rel-err "
                                   "budget is 2e-2"):
                        nc.vector.reciprocal(rrec[:, :], rden[:, :])
                    return 0

                def hop_r(no_pool=False):
                    if no_pool:
                        return None
                    nc.gpsimd.dma_start(out=rr1[0:1, :], in_=rrec[1:2, :])
                    return 0

                def bcasts(no_pool=False):
                    if no_pool:
                        return None
                    nc.gpsimd.partition_broadcast(rb_e[:, :], rrec[0:1, :])
                    nc.gpsimd.partition_broadcast(rb_o[:, :], rr1[0:1, :])
                    return 0

                def muls(no_pool=False):
                    nc.vector.tensor_mul(
                        oT_sb[0:64, hp, lsl_], stg_e[0:64, :], rb_e[:, :])
                    nc.vector.tensor_mul(
                        onorm[:, :], stg_o[0:64, :], rb_o[:, :])
                    return 0

                def hop_out(no_pool=False):
                    if no_pool:
                        return None
                    nc.gpsimd.dma_start(
                        out=oT_sb[64:128, hp, lsl_], in_=onorm[:, :])
                    return 0

                for piece in (hops_den, recips, hop_r, bcasts, muls, hop_out):
                    prep_q.append(piece)

            def attention_window(lw, qT):
                lsl = slice(lw * LW, (lw + 1) * LW)
                state["attn"] = True
                for hp in range(2):
                    po_e = ps_o.tile([128, LW], F32, tag="o")
                    po_o = ps_o.tile([128, LW], F32, tag="o")

                    # diagonal blocks first (start=True covers full width),
                    # then off-diagonal blocks.
                    order = [4 * lw + s for s in range(4)] + list(range(4 * lw))
                    nun = len(order)

                    def pv_pair(prev):
                        idx, kt, psb, q0 = prev
                        nc.tensor.matmul(
                            po_e[0:65, q0:], v_sb[:, kt, 2 * hp, :],
                            psb[:, 0, q0:],
                            start=(idx == 0), stop=(idx == nun - 1),
                        )
                        nc.tensor.matmul(
                            po_o[0:65, q0:], v_sb[:, kt, 2 * hp + 1, :],
                            psb[:, 1, q0:],
                            start=(idx == 0), stop=(idx == nun - 1),
                        )

                    prev = None
                    deficit = 0.0
                    for idx, kt in enumerate(order):
                        diag = idx < 4
                        q0 = 128 * idx if diag else 0
                        wq_ = LW - q0
                        ksb = ps_s.tile([128, 2, LW], F32, tag="s")
                        nc.tensor.matmul(
                            ksb[:, 0, q0:],
                            kT_sb[0:64, hp, kt * 128:(kt + 1) * 128],
                            qT[0:64, hp, q0:],
                            start=True, stop=True,
                        )
                        nc.tensor.matmul(
                            ksb[:, 1, q0:],
                            kT_sb[64:128, hp, kt * 128:(kt + 1) * 128],
                            qT[64:128, hp, q0:],
                            start=True, stop=True,
                        )
                        psb = p_pool.tile([128, 2, LW], MMDT, tag="p")
                        nc.scalar.activation(
                            psb[:, :, q0:], ksb[:, :, q0:],
                            func=mybir.ActivationFunctionType.Exp,
                            scale=1.0 / math.sqrt(DKH),
                        )
                        if diag:
                            # zero the 128-wide triangle of P post-exp:
                            # keep where col >= partition (key).  The PV
                            # that reads this comes one unit later, hiding
                            # the Pool latency.
                            for i in range(2):
                                nc.gpsimd.affine_select(
                                    out=psb[:, i, q0:q0 + 128],
                                    in_=psb[:, i, q0:q0 + 128],
                                    compare_op=mybir.AluOpType.is_ge,
                                    fill=0.0, base=0,
                                    pattern=[[1, 128]], channel_multiplier=-1,
                                )
                        # ledger: exp cost minus the S pair's PE cost
                        deficit += (2 * wq_ * 0.833 + 330) - 2 * wq_ * 0.4167
                        deficit += state["act_debt"]
                        state["act_debt"] = 0.0
                        npool = idx <= 2  # protect the next unit's affine
                        deficit = pull_one(deficit, no_pool=npool)
                        if prev is not None:
                            pv_pair(prev)
                            deficit -= 2 * (LW - prev[3]) * 0.4167
                            deficit = pull_one(deficit, no_pool=npool)
                        prev = (idx, kt, psb, q0)
                    deficit = pull_one(deficit, no_pool=True)
                    pv_pair(prev)
                    norm_chain(hp, lsl, po_e, po_o)
                state["attn"] = False

            # ================= main schedule =================
            xT0 = xT_pool.tile([128, 8, LW], MMDT, tag="xT")
            xvT0 = xT_pool.tile([128, 8, LW], MMDT, tag="xT")
            qT0 = qT_pool.tile([128, 2, LW], MMDT, tag="qT")

            # head phase: window-0 prep emitted directly (ACT still free,
            # so evacs go 3-way round robin until just before attention)
            for k in range(8):
                t_group('c', 0, k, xT0)()
            for m, which in ((0, 'q'), (0, 'k'), (1, 'q'), (1, 'k')):
                qk_group(which, m, 0, xT0, qT0)()
            state["act_free"] = False
            for k in range(8):
                t_group('v', 0, k, xvT0)()
            for a in range(4):
                v_group(0, a, xvT0)()

            cur_qT = qT0
            for lw in range(NW):
                # kick off next window's loads + enqueue its prep
                if lw + 1 < NW:
                    load_window(x_ctx, 'c', lw + 1)
                    load_window(x_val, 'v', lw + 1)
                    xT = xT_pool.tile([128, 8, LW], MMDT, tag="xT")
                    xvT = xT_pool.tile([128, 8, LW], MMDT, tag="xT")
                    nqT = qT_pool.tile([128, 2, LW], MMDT, tag="qT")
                    enqueue_prep(lw + 1, xT, xvT, nqT)
                attention_window(lw, cur_qT)
                if lw == NW - 1:
                    state["act_free"] = True
                # whatever prep remains (incl. this window's normalize
                # chain) must finish before outproj / the next attention.
                drain_prep()
                for lt in range(4 * lw, 4 * lw + 4):
                    for n in range(2):
                        fill_q.append(outproj_unit(lt, n))
                if lw + 1 < NW:
                    cur_qT = nqT
            while fill_q:
                fill_q.popleft()(False)

    nc.compile()
    return nc


_CACHE = {}


def _program() -> bass.Bass:
    if "nc" not in _CACHE:
        _CACHE["nc"] = build_program()
    return _CACHE["nc"]


def make_in_maps(inputs):
    ctx = np.ascontiguousarray(np.asarray(inputs["context_sequence"], np.float32))
    val = np.ascontiguousarray(np.asarray(inputs["value_sequence"], np.float32))
    Wq = np.asarray(inputs["Wq"], np.float32)
    Wk = np.asarray(inputs["Wk"], np.float32)
    Wv = np.asarray(inputs["Wv"], np.float32)
    Wo = np.asarray(inputs["Wo"], np.float32)
    bq = np.asarray(inputs["bq"], np.float32)
    bk = np.asarray(inputs["bk"], np.float32)
    bv = np.asarray(inputs["bv"], np.float32)
    in_maps = []
    for c in range(8):
        b, hg = divmod(c, 4)
        cols = slice(hg * DKC, (hg + 1) * DKC)
        in_maps.append({
            "x_ctx": ctx[b],
            "x_val": val[b],
            "wq": np.ascontiguousarray(Wq[:, cols]),
            "wk": np.ascontiguousarray(Wk[:, cols]),
            "wv": np.ascontiguousarray(Wv[:, cols]),
            "bq": np.ascontiguousarray(bq[cols]),
            "bk": np.ascontiguousarray(bk[cols]),
            "bv": np.ascontiguousarray(bv[cols]),
            "wo": np.ascontiguousarray(Wo[cols, :]),
            "ident": np.eye(128, dtype=np.float32),
        })
    return in_maps


def combine_outputs(results, bo):
    bo = np.asarray(bo, np.float32)
    outs = [np.asarray(r["out"], np.float32) for r in results]
    full = np.empty((B, L, D), np.float32)
    for b in range(B):
        acc = np.zeros((L, D), np.float64)
        for c in range(4 * b, 4 * b + 4):
            acc += outs[c]
        full[b] = (acc + bo).astype(np.float32)
    return full


def _numpy_fallback(inputs):
    """Reference semantics for a non-causal mask (the TRN kernel hardcodes
    the causal structure)."""
    ctx = np.asarray(inputs["context_sequence"], np.float32)
    val = np.asarray(inputs["value_sequence"], np.float32)
    mask = np.asarray(inputs["mask"]) != 0
    Q = (ctx @ inputs["Wq"] + inputs["bq"]).reshape(B, L, H, DKH)
    Kp = (ctx @ inputs["Wk"] + inputs["bk"]).reshape(B, L, H, DKH)
    V = (val @ inputs["Wv"] + inputs["bv"]).reshape(B, L, H, DKH)
    outs = np.zeros((B, L, D), np.float32)
    for b in range(B):
        for h in range(H):
            s = (Q[b, :, h, :] @ Kp[b, :, h, :].T) / np.sqrt(np.float32(DKH))
            s = np.where(mask, s, -np.inf)
            s = s - s.max(axis=1, keepdims=True)
            p = np.exp(s)
            p /= p.sum(axis=1, keepdims=True)
            outs[b] += (p @ V[b, :, h, :]) @ np.asarray(inputs["Wo"])[h * DKH:(h + 1) * DKH, :]
    return outs + np.asarray(inputs["bo"], np.float32)


def kernel(**inputs) -> np.ndarray:
    mask = np.asarray(inputs["mask"])
    if not np.array_equal(mask != 0, np.tril(np.ones((L, L), bool))):
        return _numpy_fallback(inputs)
    nc = _program()
    in_maps = make_in_maps(inputs)
    last_err = None
    for _attempt in range(3):
        try:
            res = run_bass_kernel_spmd(nc, in_maps, list(range(8)))
            break
        except Exception as e:  # transient NRT device wedges clear on retry
            last_err = e
    else:
        raise last_err
    return combine_outputs(res.results, inputs["bo"])


if __name__ == "__main__":
    rng = np.random.default_rng(0)
    demo = {
        "context_sequence": rng.normal(size=(B, L, D)).astype(np.float32),
        "value_sequence": rng.normal(size=(B, L, D)).astype(np.float32),
        "mask": np.tril(np.ones((L, L), np.int32)),
        **{f"W{n}": (rng.normal(size=(D, D)) / 32).astype(np.float32) for n in "qkvo"},
        **{f"b{n}": (rng.normal(size=(D,)) / 32).astype(np.float32) for n in "qkvo"},
    }
    out = kernel(**demo)
    print(out.shape, out.dtype)
